# revision 1
# baseline (speedup 1.0000x reference)
"""MinGRU cell kernel for Trainium2 (8 NeuronCores, data-parallel over batch).

Computes, for x:[B,T,D], motion_mag:[B,T]:
    tau = 1 + softplus(alpha) * sigmoid(mw*mm + mb)        (per b,t)
    z   = sigmoid((x @ Wz^T + bz) / tau)                   (B,T,H)
    ht  = x @ Wh^T + bh                                    (B,T,H)
    h_t = (1-z_t)*h_{t-1} + z_t*ht_t   (scan over t, h_0=0)

Strategy:
  - Shard B=32 across 8 cores (4 per core). Weights replicated.
  - On-chip layout: h on partitions, t on the free dim, so the recurrence is
    a HW tensor_tensor_scan per [128h, 512t] tile, carried across t-tiles via
    initial=prev[:, -1:].
  - Projections: lhsT = W^T chunks (stationary), rhs = x^T chunks (moving),
    float32r (full PE rate, near-fp32 accuracy, fp32 PSUM accumulation).
  - tau: 1/tau computed on host, DMA-broadcast across partitions per block;
    folded in via one fused scalar_tensor_tensor: u = (zpre + bz) * invtau.
  - z = sigmoid(u), a = sigmoid(-u) = 1-z on ACT; b = (hpre + bh) * z on DVE.
  - Host pre-transposes x to [d, b*t] per core and un-transposes the output.
"""

import sys

import numpy as np

if "/opt/trn_rl_repo" not in sys.path:
    sys.path.insert(0, "/opt/trn_rl_repo")

B, T, D, H = 32, 2048, 512, 512
NCORES = 8
BL = B // NCORES            # batch per core = 4
TBLK = 1024                 # t-columns per block (2 psum banks)
MMN = 512                   # matmul free-dim (1 psum bank)
NTB = T // TBLK             # 2 t-blocks per sample
DC = D // 128               # 4 contraction chunks
HC = H // 128               # 4 h partition chunks
BT = BL * T                 # 8192 columns per core

_CACHE = {}


def _build_nc(bz0=None, bh0=None):
    import concourse.bass as bass
    import concourse.bacc as bacc
    import concourse.mybir as mybir
    import concourse.tile as tile
    from contextlib import ExitStack

    f32 = mybir.dt.float32
    f32r = mybir.dt.float32r
    AF = mybir.ActivationFunctionType
    OP = mybir.AluOpType

    nc = bacc.Bacc("TRN2", target_bir_lowering=False, debug=False)

    xt_ext = nc.declare_dram_parameter("xt", [DC, 128, BT], f32r, isOutput=False)
    wzt_ext = nc.declare_dram_parameter("wzt", [HC, 128, DC, 128], f32r, isOutput=False)
    wht_ext = nc.declare_dram_parameter("wht", [HC, 128, DC, 128], f32r, isOutput=False)
    bz_ext = nc.declare_dram_parameter("bz", [HC, 128, 1], f32, isOutput=False)
    bh_ext = nc.declare_dram_parameter("bh", [HC, 128, 1], f32, isOutput=False)
    itau_ext = nc.declare_dram_parameter("invtau", [BL, 1, T], f32, isOutput=False)
    out_ext = nc.declare_dram_parameter("out", [BL, HC, 128, T], f32, isOutput=True)

    with tile.TileContext(nc) as tc, ExitStack() as ctx:
        singles = ctx.enter_context(tc.tile_pool(name="singles", bufs=1))
        x_pool = ctx.enter_context(tc.tile_pool(name="x", bufs=3))
        j_pool = ctx.enter_context(tc.tile_pool(name="j", bufs=3))
        psum = ctx.enter_context(tc.tile_pool(name="psum", bufs=2, space="PSUM"))
        work = ctx.enter_context(tc.tile_pool(name="work", bufs=4))
        ab_pool = ctx.enter_context(tc.tile_pool(name="ab", bufs=4))
        h_pool = ctx.enter_context(tc.tile_pool(name="h", bufs=8))

        # Weights are hc-major in DRAM: the first matmul group (hc=0) only
        # needs a 256KB DMA. First block's x arrives as 512-col halves so the
        # first 8-matmul group is gated on ~1.3MB instead of 3MB.
        wz_hc, wh_hc = [None] * HC, [None] * HC
        xs0h = [[None] * DC for _ in range(2)]
        wz_hc[0] = singles.tile([128, DC * 128], f32r, tag="wzhc0", name="wzhc0")
        nc.sync.dma_start(out=wz_hc[0][:], in_=wzt_ext[0])
        for dc in range(DC):
            xt = x_pool.tile([128, MMN], f32r, tag=f"x{dc}", name=f"x0a_{dc}")
            nc.sync.dma_start(out=xt[:], in_=xt_ext[dc, :, 0:MMN])
            xs0h[0][dc] = xt
        wh_hc[0] = singles.tile([128, DC * 128], f32r, tag="whhc0", name="whhc0")
        nc.sync.dma_start(out=wh_hc[0][:], in_=wht_ext[0])
        for dc in range(DC):
            xt = x_pool.tile([128, MMN], f32r, tag=f"x{dc}", name=f"x0b_{dc}")
            nc.sync.dma_start(out=xt[:], in_=xt_ext[dc, :, MMN:TBLK])
            xs0h[1][dc] = xt
        for hc in range(1, HC):
            w = singles.tile([128, DC * 128], f32r, tag=f"wzhc{hc}", name=f"wzhc{hc}")
            nc.sync.dma_start(out=w[:], in_=wzt_ext[hc])
            wz_hc[hc] = w
            w = singles.tile([128, DC * 128], f32r, tag=f"whhc{hc}", name=f"whhc{hc}")
            nc.sync.dma_start(out=w[:], in_=wht_ext[hc])
            wh_hc[hc] = w
        # gpsimd queue: the first block's 1/tau halves go first; bias columns
        # are DMA'd only when non-uniform (uniform biases ride as immediates).
        jt0 = j_pool.tile([128, TBLK], f32, tag="J", name="jt0")
        for half in range(2):
            iv0 = itau_ext[0, 0, half * MMN:(half + 1) * MMN]
            iv0_b = bass.AP(
                tensor=iv0.tensor, offset=iv0.offset, ap=[[0, 128]] + list(iv0.ap)
            )
            nc.gpsimd.dma_start(out=jt0[:, half * MMN:(half + 1) * MMN], in_=iv0_b)
        bz_col = [bz0] * HC
        bh_col = [bh0] * HC
        if bz0 is None:
            bz_col = []
            for hc in range(HC):
                bzc = singles.tile([128, 1], f32, tag=f"bz{hc}", name=f"bzc{hc}")
                nc.gpsimd.dma_start(out=bzc[:], in_=bz_ext[hc])
                bz_col.append(bzc[:])
        if bh0 is None:
            bh_col = []
            for hc in range(HC):
                bhc = singles.tile([128, 1], f32, tag=f"bh{hc}", name=f"bhc{hc}")
                nc.gpsimd.dma_start(out=bhc[:], in_=bh_ext[hc])
                bh_col.append(bhc[:])

        h_prev = [[None] * HC for _ in range(BL)]

        for b in range(BL):
            for tb in range(NTB):
                bt0 = b * T + tb * TBLK
                ts = slice(tb * TBLK, (tb + 1) * TBLK)
                first_blk = (b == 0 and tb == 0)
                if first_blk:
                    xs = None
                else:
                    xs = []
                    for dc in range(DC):
                        xt = x_pool.tile([128, TBLK], f32r, tag=f"x{dc}")
                        nc.sync.dma_start(
                            out=xt[:], in_=xt_ext[dc, :, bt0:bt0 + TBLK]
                        )
                        xs.append(xt)
                # Broadcast 1/tau row across all 128 partitions.
                if b == 0 and tb == 0:
                    jt = jt0
                else:
                    jt = j_pool.tile([128, TBLK], f32, tag="J")
                    iv = itau_ext[b, 0, tb * TBLK:(tb + 1) * TBLK]
                    iv_b = bass.AP(
                        tensor=iv.tensor, offset=iv.offset,
                        ap=[[0, 128]] + list(iv.ap),
                    )
                    nc.gpsimd.dma_start(out=jt[:], in_=iv_b)

                for hc in range(HC):
                    # First block's hc0 and the very last chain run per
                    # 512-col half: shorter pipeline ramp in, and the final
                    # out-DMA halves and overlaps the last half-scan.
                    last_blk = (b == BL - 1 and tb == NTB - 1 and hc == HC - 1)
                    nsub = 2 if ((first_blk and hc == 0) or last_blk) else 1
                    width = TBLK // nsub

                    u = work.tile([128, TBLK], f32, tag="u")
                    z = work.tile([128, TBLK], f32, tag="z")
                    a = ab_pool.tile([128, TBLK], f32, tag="a")
                    bb = ab_pool.tile([128, TBLK], f32, tag="b")
                    h = h_pool.tile([128, TBLK], f32, tag="h")

                    for sub in range(nsub):
                        ssl = slice(sub * width, (sub + 1) * width)
                        zq = psum.tile([128, width], f32, tag="zq")
                        hq = psum.tile([128, width], f32, tag="hq")
                        for half in range(width // MMN):
                            h0 = sub * width // MMN + half
                            csl = slice(h0 * MMN, (h0 + 1) * MMN)
                            psl = slice(half * MMN, (half + 1) * MMN)
                            for dc in range(DC):
                                rhs = (xs0h[h0][dc][:] if first_blk
                                       else xs[dc][:, csl])
                                nc.tensor.matmul(
                                    zq[:, psl],
                                    lhsT=wz_hc[hc][:, dc * 128:(dc + 1) * 128],
                                    rhs=rhs,
                                    start=(dc == 0),
                                    stop=(dc == DC - 1),
                                )
                        for half in range(width // MMN):
                            h0 = sub * width // MMN + half
                            csl = slice(h0 * MMN, (h0 + 1) * MMN)
                            psl = slice(half * MMN, (half + 1) * MMN)
                            for dc in range(DC):
                                rhs = (xs0h[h0][dc][:] if first_blk
                                       else xs[dc][:, csl])
                                nc.tensor.matmul(
                                    hq[:, psl],
                                    lhsT=wh_hc[hc][:, dc * 128:(dc + 1) * 128],
                                    rhs=rhs,
                                    start=(dc == 0),
                                    stop=(dc == DC - 1),
                                )

                        # u = (zpre + bz) * invtau
                        nc.vector.scalar_tensor_tensor(
                            u[:, ssl], zq[:], bz_col[hc], jt[:, ssl],
                            op0=OP.add, op1=OP.mult,
                        )
                        nc.scalar.activation(z[:, ssl], u[:, ssl], AF.Sigmoid)
                        nc.scalar.activation(
                            a[:, ssl], u[:, ssl], AF.Sigmoid, scale=-1.0
                        )
                        # b = (hpre + bh) * z
                        nc.vector.scalar_tensor_tensor(
                            bb[:, ssl], hq[:], bh_col[hc], z[:, ssl],
                            op0=OP.add, op1=OP.mult,
                        )
                        init = (
                            (0.0 if tb == 0 else h_prev[b][hc][:, TBLK - 1:TBLK])
                            if sub == 0 else h[:, sub * width - 1:sub * width]
                        )
                        nc.vector.tensor_tensor_scan(
                            h[:, ssl], a[:, ssl], bb[:, ssl], init,
                            op0=OP.mult, op1=OP.add,
                        )
                        if nsub > 1:
                            osl = slice(tb * TBLK + sub * width,
                                        tb * TBLK + (sub + 1) * width)
                            nc.sync.dma_start(
                                out=out_ext[b, hc, :, osl], in_=h[:, ssl]
                            )
                    h_prev[b][hc] = h
                    if nsub == 1:
                        nc.sync.dma_start(out=out_ext[b, hc, :, ts], in_=h[:])

    nc.compile()
    return nc


def _prep_inputs(x, motion_mag, Wz, bz, Wh, bh, motion_weight, motion_bias, alpha):
    x = np.ascontiguousarray(np.asarray(x, dtype=np.float32))
    mm = np.asarray(motion_mag, dtype=np.float32)
    Wz = np.asarray(Wz, dtype=np.float32)
    Wh = np.asarray(Wh, dtype=np.float32)
    bz = np.asarray(bz, dtype=np.float32).reshape(HC, 128, 1)
    bh = np.asarray(bh, dtype=np.float32).reshape(HC, 128, 1)
    mw = float(np.asarray(motion_weight))
    mb = float(np.asarray(motion_bias))
    al = float(np.asarray(alpha))

    a_sp = float(np.log1p(np.exp(al)))  # softplus(alpha)
    sig = 1.0 / (1.0 + np.exp(-(mw * mm + mb)))
    invtau = (1.0 / (1.0 + a_sp * sig)).astype(np.float32)

    wzt = np.ascontiguousarray(
        Wz.T.reshape(DC, 128, HC, 128).transpose(2, 1, 0, 3))
    wht = np.ascontiguousarray(
        Wh.T.reshape(DC, 128, HC, 128).transpose(2, 1, 0, 3))

    in_maps = []
    for c in range(NCORES):
        xl = x[c * BL:(c + 1) * BL].reshape(BL * T, D)
        xt = np.ascontiguousarray(xl.T).reshape(DC, 128, BT)
        in_maps.append({
            "xt": xt,
            "wzt": wzt,
            "wht": wht,
            "bz": bz,
            "bh": bh,
            "invtau": np.ascontiguousarray(
                invtau[c * BL:(c + 1) * BL]).reshape(BL, 1, T),
        })
    return in_maps


def _assemble(results):
    outs = []
    for c in range(NCORES):
        o = results[c]["out"]  # [BL, HC, 128, T]
        o = np.transpose(o, (0, 3, 1, 2)).reshape(BL, T, H)
        outs.append(o)
    return np.ascontiguousarray(np.concatenate(outs, axis=0))


def _run(inputs, trace=False):
    from concourse.bass_utils import run_bass_kernel_spmd

    bza = np.asarray(inputs["bz"], dtype=np.float32).reshape(-1)
    bha = np.asarray(inputs["bh"], dtype=np.float32).reshape(-1)
    bz0 = float(bza[0]) if np.all(bza == bza[0]) else None
    bh0 = float(bha[0]) if np.all(bha == bha[0]) else None
    key = ("nc", bz0, bh0)
    if key not in _CACHE:
        _CACHE[key] = _build_nc(bz0, bh0)
    nc = _CACHE[key]
    in_maps = _prep_inputs(**inputs)
    res = run_bass_kernel_spmd(nc, in_maps, list(range(NCORES)), trace=trace)
    return _assemble(res.results), res


def kernel(**inputs):
    out, _ = _run(inputs, trace=False)
    return out



# revision 4
# speedup vs baseline: 1.2273x; 1.2273x over previous
"""MinGRU cell kernel for Trainium2 (8 NeuronCores, data-parallel over batch).

Computes, for x:[B,T,D], motion_mag:[B,T]:
    tau = 1 + softplus(alpha) * sigmoid(mw*mm + mb)        (per b,t)
    z   = sigmoid((x @ Wz^T + bz) / tau)                   (B,T,H)
    q   = x @ Wh^T (+ bh)                                  (B,T,H)
    h_t = (1-z_t)*h_{t-1} + z_t*q_t   (scan over t, h_0=0)

Strategy (vs the stock-op baseline at ~182us):
  - bf16 matmuls and bf16 HBM I/O: halves DMA traffic and SBUF footprint;
    PE rate is identical to fp32r (1 col/cycle). End-to-end rel err ~4e-3
    (gate 2e-2).
  - The whole elementwise recurrence tail is ONE custom DVE instruction
    (MINGRU_SCAN_ANT, hand-written uOp program): it consumes the gate z
    (SBUF bf16) and the candidate q (PSUM fp32) directly and computes
    h = (1-z)*h_prev + z*q at 1 element/cycle by interleaving the two
    hc-chunk streams of an hc-pair (stock tensor_tensor_scan runs at 2
    cycles/element and needs two extra DVE passes + an extra ACT pass to
    form (1-z) and z*q). DVE busy drops ~146us -> ~75us.
  - bh is folded out via h' = h - bh: the scan runs on raw q with initial
    carry -bh, and bh is re-added on the host (bh==0 here, so it's free).
  - Per (b, tb, pair) group: 16 z-matmuls -> one 4-bank PSUM tile, 2
    STT-u (u=(zpre+bz)*invtau, written column-interleaved), 1 sigmoid
    over the interleaved pair, 16 q-matmuls -> second 4-bank PSUM tile,
    1 fused scan, 1 out-DMA. Two 4-bank PSUM tiles ping-pong.
"""

import sys

import numpy as np

if "/opt/trn_rl_repo" not in sys.path:
    sys.path.insert(0, "/opt/trn_rl_repo")

import ml_dtypes

B, T, D, H = 32, 2048, 512, 512
NCORES = 8
BL = B // NCORES            # batch per core = 4
TBLK = 1024                 # t-columns per group
NTB = T // TBLK             # 2 t-blocks per sample
MMN = 512                   # matmul free-dim (1 psum bank)
DC = D // 128               # 4 contraction chunks
HC = H // 128               # 4 h partition chunks
PAIRS = HC // 2             # 2 hc-pairs (interleaved scan streams)
BT = BL * T                 # 8192 columns per core

_CACHE = {}

# --------------------------------------------------------------------------- #
# Custom DVE op: fused interleaved minGRU scan.
#
# Over a stream of 2*N elements interleaving two recurrences A (even k) and
# B (odd k):
#     h_k = (1 - z_k) * h_{k-2} + z_k * q_k     h_{-2}=s0[p], h_{-1}=s1[p]
#
# Datapath (8 stages, v3/TRN2):
#   inputs: lane0 = SRC_0 (q), lane1 = SRC_1 (z), lane2 = ONE_F32
#   stage 0: d = q * z           ; carry z (lane1), 1.0 (lane2)
#   stage 1: e = 1.0 - z         ; capture d -> lane3
#   stage 2: m = e * h_prev      ; h_prev via NEXT_ALU_OUT_A/B (stage-3 flop)
#   stage 3: h = m + d           ; a-flop (stream A) / b-flop (stream B)
#   stages 4-7: BYPASS chain to the write port.
#
# Each stream's running h lives in its own stage-3 flop, maintained by two
# alternating steady uOps (uOp transitions are zero-cost), which makes the
# recurrence immune to pipeline stalls: a flop holds its last value until
# the next element of the SAME stream rewrites it. Two 1-cycle
# non-consuming seed uOps preload the flops from CONST_0/CONST_1 (the
# [P,1] h-init carries). Measured: 2339ns per 2048-element instruction
# (1.14 cyc/elem); exact vs numpy in fp32.
# --------------------------------------------------------------------------- #


def _define_mingru_scan():
    from concourse.dve_ops import (
        OPS,
        _SUB_OPCODE_FOR_NAME,
        CUSTOM_DVE_SPECS,
        DveOp,
    )
    from concourse.dve_spec import C0, C1, Spec, Src0, Src1
    from concourse.dve_uop import (
        ENABLE,
        AluInp,
        AluOp,
        DelayInp,
        DveOpSpec,
        InpSel,
        OutPath,
        OutSel,
        Trigger,
        UopConfig,
    )

    name = "MINGRU_SCAN_ANT"
    if name in _SUB_OPCODE_FOR_NAME:
        return next(op for op in OPS if op.name == name)

    def _steady(stream_b, other_idx):
        u = UopConfig()
        u.enable_input(InpSel.SRC_0, 1)     # lane0 = q
        u.enable_input(InpSel.SRC_1, 2)     # lane1 = z
        u.enable_input(InpSel.ONE_F32, 3)   # lane2 = 1.0
        dp = u.datapath_config
        dp[0].enable_alu(AluOp.MULTIPLY, AluInp.PREV_DELAY_0, AluInp.PREV_DELAY_1)
        dp[0].pass_through_delay(1, 2)
        dp[1].enable_alu(AluOp.SUBTRACT, AluInp.PREV_DELAY_2, AluInp.PREV_DELAY_1)
        dp[1].enable_delay_from_src(DelayInp.PREV_ALU_OUT, 3)
        dp[2].enable_alu(
            AluOp.MULTIPLY,
            AluInp.PREV_ALU_OUT,
            AluInp.NEXT_ALU_OUT_B if stream_b else AluInp.NEXT_ALU_OUT_A,
        )
        dp[2].pass_through_delay(3)
        dp[3].enable_alu(AluOp.ADD, AluInp.PREV_ALU_OUT, AluInp.PREV_DELAY_3)
        if stream_b:
            dp[3].alu_out_b_enable = ENABLE
        else:
            dp[3].alu_out_a_enable = ENABLE
        for s in range(4, 8):
            dp[s].pass_through_alu()
        u.enable_output(OutSel.ALU_OUT, OutPath.WR0_LO)
        u.require_inp0 = ENABLE
        u.require_inp1 = ENABLE
        u.repeat_count = 1
        u.trigger = (Trigger.SRC_TENSOR_DONE, Trigger.COUNT, Trigger.NONE)
        u.next_uop = (0, other_idx, 0)
        return u

    def _seed(const, stream_b, next_idx):
        u = UopConfig()
        u.enable_input(const, 1)
        dp = u.datapath_config
        dp[0].enable_alu(AluOp.BYPASS, AluInp.PREV_DELAY_0, AluInp.PREV_DELAY_0)
        for s in range(1, 8):
            dp[s].pass_through_alu()
        if stream_b:
            dp[3].alu_out_b_enable = ENABLE
        else:
            dp[3].alu_out_a_enable = ENABLE
        u.repeat_count = 1
        u.trigger = (Trigger.COUNT, Trigger.NONE, Trigger.NONE)
        u.next_uop = (next_idx, 0, 0)
        return u

    def _reference(in0, in1, s0, s1, imm2):
        P = in0.shape[0]
        q = np.asarray(in0, np.float32).reshape(P, -1)
        z = np.asarray(in1, np.float32).reshape(P, -1)
        n2 = q.shape[1]
        h = np.empty((P, n2), np.float32)
        prev = [
            np.broadcast_to(np.asarray(s0, np.float32).reshape(-1), (P,)).copy(),
            np.broadcast_to(np.asarray(s1, np.float32).reshape(-1), (P,)).copy(),
        ]
        for k in range(n2):
            s = k & 1
            prev[s] = (1.0 - z[:, k]) * prev[s] + z[:, k] * q[:, k]
            h[:, k] = prev[s]
        return h

    class _HandWrittenDveOp(DveOp):
        def compile(self, ver):
            assert ver == "v3", f"{name} only authored for v3/TRN2, got {ver}"
            s = DveOpSpec(
                name=self.name,
                opcode=_SUB_OPCODE_FOR_NAME[self.name],
                uops=[
                    _seed(InpSel.CONST_0, False, 1),
                    _seed(InpSel.CONST_1, True, 2),
                    _steady(False, 3),
                    _steady(True, 2),
                ],
                rd1_en=True,
            )
            s.validate(ver)
            return s

    op = _HandWrittenDveOp(
        name,
        Spec(body=Src0 * Src1 + C0 + C1, reference=_reference),
        subdim=False,
        uops_sha={},
    )
    row = max(_SUB_OPCODE_FOR_NAME.values()) + 1
    assert row < 0x20, f"no free opcode-table row for {name}"
    _SUB_OPCODE_FOR_NAME[name] = row
    OPS.append(op)
    CUSTOM_DVE_SPECS[name] = op.spec
    return op


def _build_nc():
    import concourse.bass as bass
    import concourse.bacc as bacc
    import concourse.mybir as mybir
    import concourse.tile as tile
    from contextlib import ExitStack

    MINGRU_SCAN = _define_mingru_scan()

    f32 = mybir.dt.float32
    bf16 = mybir.dt.bfloat16
    AF = mybir.ActivationFunctionType
    OP = mybir.AluOpType

    nc = bacc.Bacc("TRN2", target_bir_lowering=False, debug=False)

    xt_ext = nc.declare_dram_parameter("xt", [DC, 128, BT], bf16, isOutput=False)
    wzt_ext = nc.declare_dram_parameter("wzt", [HC, 128, DC, 128], bf16, isOutput=False)
    wht_ext = nc.declare_dram_parameter("wht", [HC, 128, DC, 128], bf16, isOutput=False)
    bz_ext = nc.declare_dram_parameter("bz", [HC, 128, 1], f32, isOutput=False)
    nbh_ext = nc.declare_dram_parameter("negbh", [HC, 128, 1], f32, isOutput=False)
    itau_ext = nc.declare_dram_parameter("invtau", [BL, 1, T], bf16, isOutput=False)
    # per (b, pair, tb): [128, 2*TBLK] column-interleaved (A=even, B=odd)
    out_ext = nc.declare_dram_parameter(
        "out", [BL, PAIRS, NTB, 128, 2 * TBLK], bf16, isOutput=True
    )

    with tile.TileContext(nc) as tc, ExitStack() as ctx:
        singles = ctx.enter_context(tc.tile_pool(name="singles", bufs=1))
        x_pool = ctx.enter_context(tc.tile_pool(name="x", bufs=2))
        j_pool = ctx.enter_context(tc.tile_pool(name="j", bufs=2))
        psum = ctx.enter_context(tc.tile_pool(name="psum", bufs=2, space="PSUM"))
        u_pool = ctx.enter_context(tc.tile_pool(name="u", bufs=2))
        z_pool = ctx.enter_context(tc.tile_pool(name="z", bufs=2))
        h_pool = ctx.enter_context(tc.tile_pool(name="h", bufs=3))
        c_pool = ctx.enter_context(tc.tile_pool(name="carry", bufs=2))

        # Weights, hc-major so the first pair's chunks land first.
        wz_hc, wh_hc = [None] * HC, [None] * HC
        for hc in range(HC):
            w = singles.tile([128, DC * 128], bf16, name=f"wzhc{hc}")
            nc.sync.dma_start(out=w[:], in_=wzt_ext[hc])
            wz_hc[hc] = w
            w = singles.tile([128, DC * 128], bf16, name=f"whhc{hc}")
            nc.sync.dma_start(out=w[:], in_=wht_ext[hc])
            wh_hc[hc] = w
        bz_col, nbh_col = [], []
        for hc in range(HC):
            c = singles.tile([128, 1], f32, name=f"bzc{hc}")
            nc.gpsimd.dma_start(out=c[:], in_=bz_ext[hc])
            bz_col.append(c)
            c = singles.tile([128, 1], f32, name=f"nbhc{hc}")
            nc.gpsimd.dma_start(out=c[:], in_=nbh_ext[hc])
            nbh_col.append(c)

        # carry[(pair, stream)] = [128,1] AP with h' of the last processed
        # column for that hc chunk (None until tb>0).
        carry = {}

        for b in range(BL):
            for tb in range(NTB):
                bt0 = b * T + tb * TBLK
                xs = []
                for dc in range(DC):
                    xt = x_pool.tile([128, TBLK], bf16, tag=f"x{dc}")
                    nc.sync.dma_start(out=xt[:], in_=xt_ext[dc, :, bt0:bt0 + TBLK])
                    xs.append(xt)
                jt = j_pool.tile([128, TBLK], bf16, tag="J")
                iv = itau_ext[b, 0, tb * TBLK:(tb + 1) * TBLK]
                iv_b = bass.AP(
                    tensor=iv.tensor, offset=iv.offset, ap=[[0, 128]] + list(iv.ap)
                )
                nc.gpsimd.dma_start(out=jt[:], in_=iv_b)

                for pair in range(PAIRS):
                    hcA, hcB = 2 * pair, 2 * pair + 1

                    # z-preactivations for both streams: 4 psum banks
                    zp = psum.tile([128, 2 * TBLK], f32, tag="zq")
                    for s, hc in enumerate((hcA, hcB)):
                        for dc in range(DC):
                            for half in range(TBLK // MMN):
                                csl = slice(half * MMN, (half + 1) * MMN)
                                psl = slice(
                                    s * TBLK + half * MMN,
                                    s * TBLK + (half + 1) * MMN,
                                )
                                nc.tensor.matmul(
                                    zp[:, psl],
                                    lhsT=wz_hc[hc][:, dc * 128:(dc + 1) * 128],
                                    rhs=xs[dc][:, csl],
                                    start=(dc == 0),
                                    stop=(dc == DC - 1),
                                )

                    # u = (zpre + bz) * invtau, written column-interleaved
                    u = u_pool.tile([128, 2 * TBLK], bf16, tag="u")
                    for s, hc in enumerate((hcA, hcB)):
                        u_int = bass.AP(
                            tensor=u[:].tensor,
                            offset=u[:].offset + s,
                            ap=[list(u[:].ap[0]), [2, TBLK]],
                        )
                        nc.vector.scalar_tensor_tensor(
                            u_int,
                            zp[:, s * TBLK:(s + 1) * TBLK],
                            bz_col[hc][:],
                            jt[:],
                            op0=OP.add,
                            op1=OP.mult,
                        )

                    z = z_pool.tile([128, 2 * TBLK], bf16, tag="z")
                    nc.scalar.activation(z[:], u[:], AF.Sigmoid)

                    # candidate q for both streams: the other 4 psum banks
                    qp = psum.tile([128, 2 * TBLK], f32, tag="zq")
                    for s, hc in enumerate((hcA, hcB)):
                        for dc in range(DC):
                            for half in range(TBLK // MMN):
                                csl = slice(half * MMN, (half + 1) * MMN)
                                psl = slice(
                                    s * TBLK + half * MMN,
                                    s * TBLK + (half + 1) * MMN,
                                )
                                nc.tensor.matmul(
                                    qp[:, psl],
                                    lhsT=wh_hc[hc][:, dc * 128:(dc + 1) * 128],
                                    rhs=xs[dc][:, csl],
                                    start=(dc == 0),
                                    stop=(dc == DC - 1),
                                )

                    # fused interleaved scan: h' = (1-z) h'_prev + z q
                    qa = qp[:, 0:TBLK]
                    q_pair = bass.AP(
                        tensor=qa.tensor, offset=qa.offset,
                        ap=list(qa.ap) + [[TBLK, 2]],
                    )
                    h = h_pool.tile([128, 2 * TBLK], bf16, tag="h")
                    if tb == 0:
                        s0, s1 = nbh_col[hcA][:], nbh_col[hcB][:]
                    else:
                        s0, s1 = carry[(pair, 0)], carry[(pair, 1)]
                    nc.vector._custom_dve(
                        MINGRU_SCAN,
                        out=h[:], in0=q_pair, in1=z[:], s0=s0, s1=s1,
                    )
                    if tb < NTB - 1:
                        # custom-DVE scalar reads must be fp32: stage the
                        # last column pair through a small fp32 tile (ACT)
                        ct = c_pool.tile([128, 2], f32, tag=f"c{pair}")
                        nc.scalar.copy(ct[:], h[:, 2 * TBLK - 2:2 * TBLK])
                        carry[(pair, 0)] = ct[:, 0:1]
                        carry[(pair, 1)] = ct[:, 1:2]

                    nc.sync.dma_start(out=out_ext[b, pair, tb], in_=h[:])

    nc.compile()
    return nc


def _prep_inputs(x, motion_mag, Wz, bz, Wh, bh, motion_weight, motion_bias, alpha):
    bf = ml_dtypes.bfloat16
    x = np.asarray(x, dtype=np.float32)
    mm = np.asarray(motion_mag, dtype=np.float32)
    Wz = np.asarray(Wz, dtype=np.float32)
    Wh = np.asarray(Wh, dtype=np.float32)
    bz = np.asarray(bz, dtype=np.float32).reshape(HC, 128, 1)
    bh = np.asarray(bh, dtype=np.float32).reshape(HC, 128, 1)
    mw = float(np.asarray(motion_weight))
    mb = float(np.asarray(motion_bias))
    al = float(np.asarray(alpha))

    a_sp = float(np.log1p(np.exp(al)))  # softplus(alpha)
    sig = 1.0 / (1.0 + np.exp(-(mw * mm + mb)))
    invtau = (1.0 / (1.0 + a_sp * sig)).astype(bf)

    wzt = np.ascontiguousarray(
        Wz.T.reshape(DC, 128, HC, 128).transpose(2, 1, 0, 3)).astype(bf)
    wht = np.ascontiguousarray(
        Wh.T.reshape(DC, 128, HC, 128).transpose(2, 1, 0, 3)).astype(bf)

    in_maps = []
    for c in range(NCORES):
        xl = x[c * BL:(c + 1) * BL].reshape(BL * T, D)
        xt = np.ascontiguousarray(xl.T).reshape(DC, 128, BT).astype(bf)
        in_maps.append({
            "xt": xt,
            "wzt": wzt,
            "wht": wht,
            "bz": bz,
            "negbh": -bh,
            "invtau": np.ascontiguousarray(
                invtau[c * BL:(c + 1) * BL]).reshape(BL, 1, T),
        })
    return in_maps, bh


def _assemble(results, bh):
    outs = []
    for c in range(NCORES):
        o = np.asarray(results[c]["out"], dtype=np.float32)
        # [BL, PAIRS, NTB, 128, 2*TBLK] -> [BL, T, H]
        o = o.reshape(BL, PAIRS, NTB, 128, TBLK, 2)
        o = np.transpose(o, (0, 2, 4, 1, 5, 3)).reshape(BL, T, H)
        outs.append(o)
    full = np.ascontiguousarray(np.concatenate(outs, axis=0))
    bhf = bh.reshape(H)
    if np.any(bhf):
        full += bhf
    return full


def _run(inputs, trace=False):
    from concourse.bass_utils import run_bass_kernel_spmd

    if "nc" not in _CACHE:
        _CACHE["nc"] = _build_nc()
    nc = _CACHE["nc"]
    in_maps, bh = _prep_inputs(**inputs)
    res = run_bass_kernel_spmd(nc, in_maps, list(range(NCORES)), trace=trace)
    return _assemble(res.results, bh), res


def kernel(**inputs):
    out, _ = _run(inputs, trace=False)
    return out
